# revision 12
# baseline (speedup 1.0000x reference)
"""Chamfer distance loss kernel for Trainium2 (8 NeuronCores).

Problem: template/source (4, 8192, 3) f32. For each batch b:
  d[n,m] = |t_n|^2 - 2 t_n.s_m + |s_m|^2
  loss_b = mean_n min_m d + mean_m min_n d ; output = mean_b loss_b (scalar).

Sharding: core c handles (batch = c//2, template-row-half = c%2):
4096 template rows x all 8192 source points. The distance matrix tile
[128 rows x 512 cols] is produced directly in PSUM by a single K=5
augmented matmul: lhsT rows = [t0,t1,t2,|t|^2,1], rhs rows =
[-2 s0,-2 s1,-2 s2, 1, |s|^2]. ScalarE evacuates PSUM to SBUF as fp16
(min-selection in fp16 is numerically safe here: ~2.5e-6 final rel err),
VectorE accumulates row-wise and column-wise minima in fp16 (2x packed
mode), and the per-core partials (row-min sums, 16-partition-folded
column minima) are combined on the host.
"""
import os
import sys

sys.path.insert(0, "/opt/trn_rl_repo")

from contextlib import ExitStack

import numpy as np

import concourse.bass as bass
import concourse.tile as tile
from concourse import mybir
from concourse.bass_utils import run_bass_kernel_spmd

# ---------------------------------------------------------------------------
# The walrus build in this container rejects instructions carrying more than
# one sync-wait command. After Tile scheduling, split any multi-wait
# instruction: keep the first wait on it and hoist the rest onto standalone
# EventSemaphore instructions inserted just before it (same engine, so
# per-engine program order makes the waits execute first).
import bass_rust as _br


def split_multi_waits(nc):
    n_new = 0
    for fn in nc.m.functions:
        for blk in fn.blocks:
            insts = list(blk.instructions)
            out = []
            changed = False
            for inst in insts:
                si = inst.sync_info
                waits = list(si.on_wait) if si is not None and si.on_wait else []
                if len(waits) > 1:
                    for w in waits[:-1]:
                        ev = _br.InstEventSemaphore(
                            name=f"I-waitsplit-{n_new}", ins=[], outs=[]
                        )
                        n_new += 1
                        ev.engine = inst.engine
                        ev.sync_info = _br.SyncInfo(on_wait=[w], on_update=[])
                        out.append(ev)
                    si.on_wait = [waits[-1]]
                    changed = True
                out.append(inst)
            if changed:
                blk.instructions = out
# ---------------------------------------------------------------------------

import ml_dtypes

F32 = mybir.dt.float32
F32R = mybir.dt.float32r
F16 = mybir.dt.float16
BF16 = mybir.dt.bfloat16
MIN = mybir.AluOpType.min
BF16NP = ml_dtypes.bfloat16

B, N, M, D = 4, 8192, 8192, 3
R = N // 2      # template rows per core
NCORES = 8
GROUP = 2048    # psum group: 4 matmuls of 512
PFOLD = 16      # colmin partition count returned to host

# "dekker": bf16 hi/lo split matmuls, K=16 (1 cycle/row; ~2e-6 final rel err)
# "f32"   : exact fp32 matmuls, K=5 (4 cycles/row, slowest, exact)
# "f32r"  : float32r matmuls, K=5 (fast but ~tf32 precision: too coarse)
MM_MODE = os.environ.get("CHAMFER_MM_MODE", "dekker")
K_BY_MODE = {"dekker": 16, "f32": 5, "f32r": 5}
K = K_BY_MODE[MM_MODE]


def build_program(rows=R, cols=M, mm_mode=MM_MODE, split_waits=True):
    row_tiles = rows // 128
    ngroups = cols // GROUP
    k = K_BY_MODE[mm_mode]
    nc = bass.Bass("TRN2", target_bir_lowering=False, debug=False)
    mm_dt = {"dekker": BF16, "f32": F32, "f32r": F32R}[mm_mode]
    lhsT = nc.dram_tensor("lhsT_aug", [k, rows], mm_dt, kind="ExternalInput").ap()
    rhs = nc.dram_tensor("rhs_aug", [k, cols], mm_dt, kind="ExternalInput").ap()
    o_rm = nc.dram_tensor(
        "out_rowmin", [128, row_tiles], F32, kind="ExternalOutput"
    ).ap()
    o_cm = nc.dram_tensor("out_colmin", [PFOLD, cols], F16, kind="ExternalOutput").ap()

    with tile.TileContext(nc) as tc, ExitStack() as ctx:
        consts = ctx.enter_context(tc.tile_pool(name="consts", bufs=1))
        psum_pool = ctx.enter_context(tc.tile_pool(name="psum", bufs=2, space="PSUM"))
        cast_pool = ctx.enter_context(tc.tile_pool(name="cast", bufs=3))
        rowacc_pool = ctx.enter_context(tc.tile_pool(name="rowacc", bufs=2))
        accs = ctx.enter_context(tc.tile_pool(name="accs", bufs=1))

        lhsT_sb = consts.tile([k, rows], mm_dt)
        nc.sync.dma_start(lhsT_sb[:], lhsT)
        rhs_sb = consts.tile([k, cols], mm_dt)
        nc.sync.dma_start(rhs_sb[:], rhs)

        colacc = accs.tile([128, cols], F16)
        rowminb = accs.tile([128, row_tiles], F32)

        for i in range(row_tiles):
            lh = lhsT_sb[:, i * 128:(i + 1) * 128]
            rowacc = rowacc_pool.tile([128, GROUP], F16)
            for g in range(ngroups):
                ps = psum_pool.tile([128, GROUP], F32)
                for jj in range(4):
                    c0 = g * GROUP + jj * 512
                    nc.tensor.matmul(
                        ps[:, jj * 512:(jj + 1) * 512], lh,
                        rhs_sb[:, c0:c0 + 512],
                        start=True, stop=True,
                    )
                if i == 0:
                    dst = colacc[:, g * GROUP:(g + 1) * GROUP]
                    nc.scalar.copy(dst, ps[:])
                    if g == 0:
                        nc.vector.tensor_copy(rowacc[:], dst)
                    else:
                        nc.vector.tensor_tensor(rowacc[:], rowacc[:], dst, op=MIN)
                else:
                    cst = cast_pool.tile([128, GROUP], F16)
                    nc.scalar.copy(cst[:], ps[:])
                    if g == 0:
                        nc.vector.tensor_copy(rowacc[:], cst[:])
                    else:
                        nc.vector.tensor_tensor(rowacc[:], rowacc[:], cst[:], op=MIN)
                    ca = colacc[:, g * GROUP:(g + 1) * GROUP]
                    nc.vector.tensor_tensor(ca, ca, cst[:], op=MIN)
            nc.vector.tensor_reduce(
                rowminb[:, i:i + 1], rowacc[:], axis=mybir.AxisListType.X, op=MIN
            )

        # Fold colacc partitions 128 -> PFOLD. DVE lanes cannot cross
        # partitions, so shift the upper half down via SBUF->SBUF DMA first.
        scratch = accs.tile([64, cols], F16)
        hp = 64
        while hp >= PFOLD:
            nc.sync.dma_start(scratch[0:hp, :], colacc[hp:2 * hp, :])
            nc.vector.tensor_tensor(
                colacc[0:hp, :], colacc[0:hp, :], scratch[0:hp, :], op=MIN
            )
            hp //= 2

        nc.sync.dma_start(o_cm, colacc[0:PFOLD, :])
        nc.sync.dma_start(o_rm, rowminb[:])
    if split_waits:
        split_multi_waits(nc)  # CoreSim can't model the injected waits
    return nc


_program_cache = {}


def _get_program():
    key = (R, M, MM_MODE)
    if key not in _program_cache:
        _program_cache[key] = build_program()
    return _program_cache[key]


def _aug_f32(t, s):
    """K=5 fp32 augmentation: d = |t|^2 - 2 t.s + |s|^2 in one matmul."""
    rows, cols = t.shape[0], s.shape[0]
    lhsT = np.empty((5, rows), np.float32)
    lhsT[0:3] = t.T
    lhsT[3] = (t * t).sum(axis=1)
    lhsT[4] = 1.0
    rhs = np.empty((5, cols), np.float32)
    rhs[0:3] = -2.0 * s.T
    rhs[3] = 1.0
    rhs[4] = (s * s).sum(axis=1)
    return lhsT, rhs


def _aug_dekker(t, s):
    """K=16 bf16 hi/lo augmentation (Dekker split, exact to ~1e-4 abs)."""
    rows, cols = t.shape[0], s.shape[0]
    th = t.astype(BF16NP)
    tl = (t - th.astype(np.float32)).astype(BF16NP)
    sm = -2.0 * s
    sh = sm.astype(BF16NP)
    sl = (sm - sh.astype(np.float32)).astype(BF16NP)
    nt = (t * t).sum(axis=1)
    nth = nt.astype(BF16NP)
    ntl = (nt - nth.astype(np.float32)).astype(BF16NP)
    ns = (s * s).sum(axis=1)
    nsh = ns.astype(BF16NP)
    nsl = (ns - nsh.astype(np.float32)).astype(BF16NP)
    one = np.ones((), BF16NP)
    lhsT = np.empty((16, rows), BF16NP)
    lhsT[0:3] = th.T
    lhsT[3:6] = th.T
    lhsT[6:9] = tl.T
    lhsT[9:12] = tl.T
    lhsT[12] = nth
    lhsT[13] = ntl
    lhsT[14] = one
    lhsT[15] = one
    rhs = np.empty((16, cols), BF16NP)
    rhs[0:3] = sh.T
    rhs[3:6] = sl.T
    rhs[6:9] = sh.T
    rhs[9:12] = sl.T
    rhs[12] = one
    rhs[13] = one
    rhs[14] = nsh
    rhs[15] = nsl
    return lhsT, rhs


def make_in_maps(template, source, mm_mode=MM_MODE):
    template = np.asarray(template, dtype=np.float32)
    source = np.asarray(source, dtype=np.float32)
    aug = _aug_dekker if mm_mode == "dekker" else _aug_f32
    in_maps = []
    for c in range(NCORES):
        b, h = c // 2, c % 2
        t = template[b, h * R:(h + 1) * R]      # [R, 3]
        s = source[b]                            # [M, 3]
        lhsT, rhs = aug(t, s)
        in_maps.append(
            {"lhsT_aug": np.ascontiguousarray(lhsT),
             "rhs_aug": np.ascontiguousarray(rhs)}
        )
    return in_maps


last_results = None  # BassKernelResults of the most recent kernel() call


def kernel(template, source):
    global last_results
    nc = _get_program()
    in_maps = make_in_maps(template, source)
    res = run_bass_kernel_spmd(nc, in_maps, list(range(NCORES)))
    last_results = res

    per_batch = np.zeros(B, dtype=np.float64)
    for b in range(B):
        r0 = res.results[2 * b + 0]
        r1 = res.results[2 * b + 1]
        rowsum = (
            r0["out_rowmin"].astype(np.float64).sum()
            + r1["out_rowmin"].astype(np.float64).sum()
        )
        cost_p0_p1 = rowsum / N
        cm = np.minimum(
            r0["out_colmin"].astype(np.float32).min(axis=0),
            r1["out_colmin"].astype(np.float32).min(axis=0),
        )
        cost_p1_p0 = cm.astype(np.float64).mean()
        per_batch[b] = cost_p0_p1 + cost_p1_p0
    return np.float32(per_batch.mean())
